# revision 14
# baseline (speedup 1.0000x reference)
"""Trainium2 Bass kernel: causal GQA self-attention (B=2, T=2048, C=1024,
16 q-heads / 4 kv-heads, rotary + q/k RMS-norm), sharded over 8 NeuronCores
as (batch x kv-group). Self-contained: kernel(**inputs) -> np.ndarray.
"""
import sys
from contextlib import ExitStack

for p in ("/opt/trn_rl_repo", "/root/.axon_site/_ro/trn_rl_repo"):
    if p not in sys.path:
        sys.path.insert(0, p)

import numpy as np
import ml_dtypes

import concourse.bass as bass
import concourse.mybir as mybir
from concourse.tile import TileContext
from concourse.masks import make_identity

F32 = mybir.dt.float32
BF16 = mybir.dt.bfloat16
NPBF16 = ml_dtypes.bfloat16

T, C, HQ, D = 2048, 1024, 4, 64
DQ = HQ * D
TC = T // 128
KC = C // 128
NJ = T // 512
EPS = 1.1920929e-7


def _bcast_ap(sl, n, at=1):
    ap = list(sl.ap)
    ap.insert(at, [0, n])
    return bass.AP(tensor=sl.tensor, offset=sl.offset, ap=ap)


def _split_waits(nc, maxw=1):
    """Walrus in this toolchain allows 1 sem-wait per instruction; split extras
    onto preceding same-engine NoOps."""
    cnt = 0
    for f in nc.m.functions:
        for b in f.blocks:
            il = list(b.instructions)
            out = []
            changed = False
            for inst in il:
                si = inst.sync_info
                waits = list(si.on_wait) if si and si.on_wait else []
                if len(waits) > maxw:
                    chunks = [waits[i:i + maxw] for i in range(0, len(waits), maxw)]
                    for ch in chunks[:-1]:
                        cnt += 1
                        nop = mybir.InstNoOp(name=f"I-waitfix-{cnt}")
                        nop.engine = inst.engine
                        nop.sync_info = mybir.SyncInfo(on_wait=ch, on_update=[])
                        out.append(nop)
                    si.on_wait = chunks[-1]
                    inst.sync_info = si
                    changed = True
                out.append(inst)
            if changed:
                b.instructions = out
    return cnt


def _build_attn(ctx, tc, outs, ins):
    nc = tc.nc
    xT, wq, wkv, wo, cos2, sin2 = (
        ins["xT"], ins["wq"], ins["wkv"], ins["wo"], ins["cos2"], ins["sin2"])
    outT = outs["outT"]

    singles = ctx.enter_context(tc.tile_pool(name="singles", bufs=1))

    ident = singles.tile([128, 128], F32, tag="ident")
    make_identity(nc, ident)
    eps_t = singles.tile([128, 1], F32, tag="eps_t")
    nc.vector.memset(eps_t, EPS)
    # broadcast selectors: ohlo -> partitions 0-63, ohhi -> 64-127
    ohlo = singles.tile([1, 128], F32, tag="ohlo")
    nc.vector.memset(ohlo, 0.0)
    nc.vector.memset(ohlo[0:1, 0:64], 1.0)
    ohhi = singles.tile([1, 128], F32, tag="ohhi")
    nc.vector.memset(ohhi, 0.0)
    nc.vector.memset(ohhi[0:1, 64:128], 1.0)

    # per-512-column-chunk tiles so phase 2 deps are per chunk
    xsb = [singles.tile([128, KC, 512], BF16, tag=f"xsb{s}", name=f"xsb{s}") for s in range(4)]
    for s in range(4):
        nc.sync.dma_start(
            out=xsb[s],
            in_=xT[:, s * 512:(s + 1) * 512].rearrange("(a p) t -> p a t", p=128))
    wq_sb = singles.tile([128, KC, DQ], BF16, tag="wq_sb")
    nc.sync.dma_start(out=wq_sb, in_=wq.rearrange("(a p) n -> p a n", p=128))
    wkv_sb = singles.tile([128, KC, 128], BF16, tag="wkv_sb")
    nc.sync.dma_start(out=wkv_sb, in_=wkv.rearrange("(a p) n -> p a n", p=128))
    cos_sb = singles.tile([128, TC, 32], F32, tag="cos_sb")
    nc.sync.dma_start(out=cos_sb, in_=cos2.rearrange("(a p) d -> p a d", p=128))
    sin_sb = singles.tile([128, TC, 32], F32, tag="sin_sb")
    nc.sync.dma_start(out=sin_sb, in_=sin2.rearrange("(a p) d -> p a d", p=128))
    wo_sb = singles.tile([128, 2, C], BF16, tag="wo_sb")
    nc.sync.dma_start(out=wo_sb, in_=wo.rearrange("(a p) o -> p a o", p=128))

    # transposed q/k, values, attention outputs: per-512-col tiles
    qt0 = [singles.tile([128, 512], BF16, tag=f"qt0_{s}", name=f"qt0_{s}") for s in range(4)]
    qt1 = [singles.tile([128, 512], BF16, tag=f"qt1_{s}", name=f"qt1_{s}") for s in range(4)]
    kt2 = [singles.tile([128, 512], BF16, tag=f"kt2_{s}", name=f"kt2_{s}") for s in range(4)]
    yt0 = [singles.tile([128, 512], BF16, tag=f"yt0_{s}", name=f"yt0_{s}") for s in range(4)]
    yt1 = [singles.tile([128, 512], BF16, tag=f"yt1_{s}", name=f"yt1_{s}") for s in range(4)]
    qts = (qt0, qt1)
    yts = (yt0, yt1)
    v_sb = [singles.tile([128, 4, 65], BF16, tag=f"v_sb{s}", name=f"v_sb{s}") for s in range(4)]
    for s in range(4):
        nc.vector.memset(v_sb[s][:, :, 64:65], 1.0)

    # ---- Phase 1: projections + rope + rms + transposes (per supertile) ----
    with (
        tc.tile_pool(name="scratch", bufs=2) as scratch,
        tc.tile_pool(name="rtmp", bufs=2) as rtmp,
        tc.tile_pool(name="pp", bufs=2, space="PSUM") as pp,
        tc.tile_pool(name="tpp", bufs=2, space="PSUM") as tpp,
    ):
        for s in range(4):
            qps4 = pp.tile([128, 4, DQ], F32, tag="qps4")
            kvps4 = pp.tile([128, 4, 128], F32, tag="kvps4")
            for tt in range(4):
                for kc in range(KC):
                    nc.tensor.matmul(
                        qps4[:, tt, :], xsb[s][:, kc, tt * 128:(tt + 1) * 128],
                        wq_sb[:, kc, :], start=(kc == 0), stop=(kc == KC - 1))
                for kc in range(KC):
                    nc.tensor.matmul(
                        kvps4[:, tt, :], xsb[s][:, kc, tt * 128:(tt + 1) * 128],
                        wkv_sb[:, kc, :], start=(kc == 0), stop=(kc == KC - 1))
            nc.scalar.copy(v_sb[s][:, :, 0:64], kvps4[:, :, 64:128])

            # q rope: supertile batch
            q4 = scratch.tile([128, 4, DQ], F32, tag="q4")
            q4v = q4.rearrange("p t (h d) -> p t h d", h=HQ)
            x1 = qps4.rearrange("p t (h d) -> p t h d", h=HQ)[:, :, :, 0:32]
            x2 = qps4.rearrange("p t (h d) -> p t h d", h=HQ)[:, :, :, 32:64]
            cb = _bcast_ap(cos_sb[:, 4 * s:4 * s + 4, :], HQ, at=2)
            sb = _bcast_ap(sin_sb[:, 4 * s:4 * s + 4, :], HQ, at=2)
            t1 = rtmp.tile([128, 4, HQ, 32], F32, tag="t1")
            t2 = rtmp.tile([128, 4, HQ, 32], F32, tag="t2")
            t3 = rtmp.tile([128, 4, HQ, 32], F32, tag="t3")
            t4 = rtmp.tile([128, 4, HQ, 32], F32, tag="t4")
            nc.vector.tensor_mul(t1, x1, cb)
            nc.vector.tensor_mul(t2, x2, sb)
            nc.vector.tensor_add(q4v[:, :, :, 0:32], t1, t2)
            nc.vector.tensor_mul(t3, x1, sb)
            nc.vector.tensor_mul(t4, x2, cb)
            nc.vector.tensor_sub(q4v[:, :, :, 32:64], t4, t3)
            # q rms-norm
            sq = scratch.tile([128, 4, DQ], F32, tag="sq")
            nc.vector.tensor_mul(sq, q4, q4)
            mv = rtmp.tile([128, 4, HQ], F32, tag="mv")
            nc.vector.tensor_reduce(
                mv, sq.rearrange("p t (h d) -> p t h d", d=D),
                axis=mybir.AxisListType.X, op=mybir.AluOpType.add)
            sd = rtmp.tile([128, 4, HQ], F32, tag="sd")
            nc.scalar.activation(sd, mv, mybir.ActivationFunctionType.Sqrt,
                                 bias=eps_t, scale=1.0 / D)
            rq = rtmp.tile([128, 4, HQ], F32, tag="rq")
            nc.vector.reciprocal(rq, sd)
            nc.vector.tensor_mul(q4v, q4v, _bcast_ap(rq, D, at=3))

            # k rope
            kn4 = scratch.tile([128, 4, 128], F32, tag="kn4")
            kx1 = kvps4[:, :, 0:32]
            kx2 = kvps4[:, :, 32:64]
            cb2 = cos_sb[:, 4 * s:4 * s + 4, :]
            sb2 = sin_sb[:, 4 * s:4 * s + 4, :]
            u1 = rtmp.tile([128, 4, 32], F32, tag="u1")
            u2 = rtmp.tile([128, 4, 32], F32, tag="u2")
            u3 = rtmp.tile([128, 4, 32], F32, tag="u3")
            u4 = rtmp.tile([128, 4, 32], F32, tag="u4")
            nc.vector.tensor_mul(u1, kx1, cb2)
            nc.vector.tensor_mul(u2, kx2, sb2)
            nc.vector.tensor_add(kn4[:, :, 0:32], u1, u2)
            nc.vector.tensor_mul(u3, kx1, sb2)
            nc.vector.tensor_mul(u4, kx2, cb2)
            nc.vector.tensor_sub(kn4[:, :, 32:64], u4, u3)
            # k rms-norm
            sqk = scratch.tile([128, 4, 64], F32, tag="sqk")
            nc.vector.tensor_mul(sqk, kn4[:, :, 0:64], kn4[:, :, 0:64])
            mvk = rtmp.tile([128, 4], F32, tag="mvk")
            nc.vector.tensor_reduce(mvk, sqk, axis=mybir.AxisListType.X,
                                    op=mybir.AluOpType.add)
            sdk = rtmp.tile([128, 4], F32, tag="sdk")
            nc.scalar.activation(sdk, mvk, mybir.ActivationFunctionType.Sqrt,
                                 bias=eps_t, scale=1.0 / D)
            rk = rtmp.tile([128, 4], F32, tag="rk")
            nc.vector.reciprocal(rk, sdk)
            nc.vector.tensor_mul(kn4[:, :, 0:64], kn4[:, :, 0:64],
                                 _bcast_ap(rk, 64, at=2))
            # duplicate k for head-pair packing
            nc.vector.tensor_copy(kn4[:, :, 64:128], kn4[:, :, 0:64])

            # transposes into [d, t] layout (bf16)
            for tt in range(4):
                for fs in range(2):
                    tps = tpp.tile([128, 128], F32, tag="tps")
                    nc.tensor.transpose(
                        tps, q4[:, tt, fs * 128:(fs + 1) * 128], ident)
                    cp = nc.scalar.copy if (tt % 2) else nc.vector.tensor_copy
                    cp(qts[fs][s][:, tt * 128:(tt + 1) * 128], tps)
                tps2 = tpp.tile([128, 128], F32, tag="tps")
                nc.tensor.transpose(tps2, kn4[:, tt, :], ident)
                cp = nc.vector.tensor_copy if (tt % 2) else nc.scalar.copy
                cp(kt2[s][:, tt * 128:(tt + 1) * 128], tps2)

    # ---- Phase 2: attention + Phase 3: out projection (per q-block j) ----
    with (
        tc.tile_pool(name="ptp", bufs=4) as ptp,
        tc.tile_pool(name="smallp", bufs=4) as smallp,
        tc.tile_pool(name="s2p", bufs=2, space="PSUM") as s2p,
        tc.tile_pool(name="o65p", bufs=2, space="PSUM") as o65p,
        tc.tile_pool(name="opp", bufs=1, space="PSUM") as opp,
        tc.tile_pool(name="bcp", bufs=1, space="PSUM") as bcp,
        tc.tile_pool(name="osp", bufs=3) as osp,
    ):
        for j in range(NJ):
            for pair in range(2):
                o65s = []
                for hh in range(2):
                    base = hh * 64
                    tp = (base, 0) if base else None
                    o65 = o65p.tile([65, 512], F32, tag="o65")
                    o65s.append(o65)
                    nu = 2 * (j + 1)
                    for u in range(nu):
                        s2 = s2p.tile([128, 2, 512], F32, tag="s2")
                        for i2 in range(2):
                            c = 2 * u + i2
                            nc.tensor.matmul(
                                s2[:, i2, :],
                                kt2[c // 4][base:base + 64,
                                            (c % 4) * 128:(c % 4 + 1) * 128],
                                qts[pair][j][base:base + 64, :],
                                start=True, stop=True, tile_position=tp)
                        pt = ptp.tile([128, 2, 512], BF16, tag="pt")
                        nc.scalar.activation(pt, s2,
                                             mybir.ActivationFunctionType.Exp,
                                             scale=0.125)
                        if u >= 2 * j:
                            # diagonal: zero the non-causal region post-exp
                            for i2 in range(2):
                                il = 2 * u + i2 - 4 * j
                                nc.gpsimd.affine_select(
                                    out=pt[:, i2, :], in_=pt[:, i2, :],
                                    compare_op=mybir.AluOpType.is_ge, fill=0.0,
                                    base=-128 * il, pattern=[[1, 512]],
                                    channel_multiplier=-1)
                        for i2 in range(2):
                            c = 2 * u + i2
                            nc.tensor.matmul(
                                o65, v_sb[c // 4][:, c % 4, 0:65], pt[:, i2, :],
                                start=(u == 0 and i2 == 0),
                                stop=(u == nu - 1 and i2 == 1))
                # pair epilogue: copy PSUM out (freeing the o65 slots),
                # broadcast denominators via matmul, one reciprocal + one mul
                oc = smallp.tile([128, 512], F32, tag="oc")
                dns = []
                for hh in range(2):
                    nc.vector.tensor_copy(oc[hh * 64:(hh + 1) * 64, :],
                                          o65s[hh][0:64, :])
                    dn = smallp.tile([1, 512], F32, tag=f"dn{hh}",
                                     name=f"dn{hh}")
                    nc.vector.tensor_copy(dn, o65s[hh][64:65, :])
                    dns.append(dn)
                bc2 = bcp.tile([128, 512], F32, tag="bc2")
                nc.tensor.matmul(bc2, ohlo, dns[0], start=True, stop=False)
                nc.tensor.matmul(bc2, ohhi, dns[1], start=False, stop=True)
                bcs = smallp.tile([128, 512], F32, tag="bcs")
                nc.vector.reciprocal(bcs, bc2)
                nc.vector.tensor_mul(yts[pair][j], oc, bcs)
            # out projection for this q block
            for m in range(8):
                ops_ = opp.tile([128, 512], F32, tag="ops")
                for fc in range(2):
                    nc.tensor.matmul(
                        ops_, wo_sb[:, fc, m * 128:(m + 1) * 128],
                        yts[fc][j][:, :], start=(fc == 0), stop=(fc == 1))
                ot = osp.tile([128, 512], BF16, tag="ot")
                nc.vector.tensor_copy(ot, ops_)
                nc.sync.dma_start(
                    out=outT[m * 128:(m + 1) * 128, j * 512:(j + 1) * 512],
                    in_=ot)


def _build_nc():
    nc = bass.Bass("TRN2", target_bir_lowering=False, debug=False, num_devices=8)
    ins = {
        "xT": nc.dram_tensor("xT", [1024, 2048], BF16, kind="ExternalInput").ap(),
        "wq": nc.dram_tensor("wq", [1024, 256], BF16, kind="ExternalInput").ap(),
        "wkv": nc.dram_tensor("wkv", [1024, 128], BF16, kind="ExternalInput").ap(),
        "wo": nc.dram_tensor("wo", [256, 1024], BF16, kind="ExternalInput").ap(),
        "cos2": nc.dram_tensor("cos2", [2048, 32], F32, kind="ExternalInput").ap(),
        "sin2": nc.dram_tensor("sin2", [2048, 32], F32, kind="ExternalInput").ap(),
    }
    outs = {"outT": nc.dram_tensor("outT", [1024, 2048], BF16,
                                   kind="ExternalOutput").ap()}
    with TileContext(nc) as tc:
        with ExitStack() as ctx:
            _build_attn(ctx, tc, outs, ins)
    _split_waits(nc, maxw=1)
    return nc


def _shard_inputs(inputs, b, g):
    x, cos, sin = inputs["x"], inputs["cos"], inputs["sin"]
    Wq, Wk, Wv, Wo = inputs["Wq"], inputs["Wk"], inputs["Wv"], inputs["Wo"]
    qs, ks = slice(g * 256, (g + 1) * 256), slice(g * 64, (g + 1) * 64)
    return {
        "xT": np.ascontiguousarray(np.asarray(x[b]).T.astype(NPBF16)),
        "wq": np.ascontiguousarray(np.asarray(Wq[qs]).T.astype(NPBF16)),
        "wkv": np.ascontiguousarray(np.concatenate(
            [np.asarray(Wk[ks]).T, np.asarray(Wv[ks]).T], axis=1).astype(NPBF16)),
        "wo": np.ascontiguousarray(np.asarray(Wo[:, qs]).T.astype(NPBF16)),
        "cos2": np.ascontiguousarray(np.asarray(cos[0, :, 0, :]), dtype=np.float32),
        "sin2": np.ascontiguousarray(np.asarray(sin[0, :, 0, :]), dtype=np.float32),
    }


_STATE = None


def _get_state():
    global _STATE
    if _STATE is not None:
        return _STATE
    import jax
    from jax.sharding import Mesh, PartitionSpec, NamedSharding
    from jax.experimental.shard_map import shard_map
    from concourse.bass2jax import (
        _bass_exec_p, install_neuronx_cc_hook, partition_id_tensor)

    install_neuronx_cc_hook()
    nc = _build_nc()
    pname = nc.partition_id_tensor.name if nc.partition_id_tensor else None

    in_names, out_names, out_avals, zero_outs = [], [], [], []
    for alloc in nc.m.functions[0].allocations:
        if not isinstance(alloc, mybir.MemoryLocationSet):
            continue
        name = alloc.memorylocations[0].name
        if alloc.kind == "ExternalInput":
            if name != pname:
                in_names.append(name)
        elif alloc.kind == "ExternalOutput":
            out_names.append(name)
            shape = tuple(alloc.tensor_shape)
            dtype = mybir.dt.np(alloc.dtype)
            out_avals.append(jax.core.ShapedArray(shape, dtype))
            zero_outs.append(np.zeros(shape, dtype))
    n_params = len(in_names)
    all_names = in_names + out_names
    if pname is not None:
        all_names = all_names + [pname]

    def _body(*args):
        operands = list(args)
        if pname is not None:
            operands.append(partition_id_tensor())
        outs = _bass_exec_p.bind(
            *operands, out_avals=tuple(out_avals), in_names=tuple(all_names),
            out_names=tuple(out_names), lowering_input_output_aliases=(),
            sim_require_finite=True, sim_require_nnan=True, nc=nc)
        return tuple(outs)

    devices = jax.devices()[:8]
    mesh = Mesh(np.asarray(devices), ("core",))
    specs = (PartitionSpec("core"),) * (n_params + 1)
    sharded = jax.jit(shard_map(_body, mesh=mesh, in_specs=specs,
                                out_specs=(PartitionSpec("core"),),
                                check_rep=False))
    sharding = NamedSharding(mesh, PartitionSpec("core"))
    zeros = jax.device_put(
        np.zeros((8 * 1024, 2048), NPBF16), sharding)
    _STATE = dict(sharded=sharded, sharding=sharding, in_names=in_names,
                  zeros=zeros, jax=jax)
    return _STATE


def _run_device(in_maps):
    st = _get_state()
    jax = st["jax"]
    concat_in = [np.concatenate([m[n] for m in in_maps], axis=0)
                 for n in st["in_names"]]
    dev_in = [jax.device_put(a, st["sharding"]) for a in concat_in]
    out = st["sharded"](*dev_in, st["zeros"])[0]
    return np.asarray(out).reshape(8, 1024, 2048)


def kernel(**inputs) -> np.ndarray:
    inputs = {k: np.asarray(v) for k, v in inputs.items()}
    in_maps = [_shard_inputs(inputs, b, g) for b in range(2) for g in range(4)]
    arr = _run_device(in_maps).astype(np.float32)
    out = np.zeros((2, 2048, 1024), np.float32)
    for c in range(8):
        out[c // 4] += arr[c].T
    return out
